# revision 16
# baseline (speedup 1.0000x reference)
# Trainium2 Bass kernel for nn_CausalSelfAttention_58239756533763.
#
# Sharding: tensor-parallel over heads. 16 heads / 8 cores = 2 heads per
# core. Each core computes q/k/v projections for its 2 heads (all 4
# batches), attention with XL memory, and a partial output projection
# against its 128 rows of Wo. The host sums the 8 partial outputs (the
# "all-reduce" of the sharding hint, done at unshard time) and concatenates
# the per-core k/v slices into the kv_to_add_xl output.
#
# On-chip layout: activations flow in the transposed ("feature on
# partitions") layout so only k/v need on-device 128x128 PE transposes
# (for the kv output / P@V operand):
#   x^T, q^T, k^T, v^T are [feature, token]; scores are S^T = [j, i].
#   Softmax denominators come from a 64-wide ones block appended to v in
#   the P@V matmul, so the sums land on partitions 64:128 of the same
#   PSUM tile (vectorized reciprocal, no cross-partition reduce).
#   exp((S + rel) * s) = exp(S * s) * exp(rel * s): the host precomputes
#   exp(rel^T * s) in fp16 with masked (j, i) entries set to 0, so the
#   bias-add becomes an fp16 multiply and causal masking is exact.
# Matmuls run in fp16 (one PE pass, ~5e-4 quantization) with fp32 PSUM
# accumulation; softmax exp runs on the Scalar engine in fp32.

import numpy as np

B, T, MXL, E, H, D = 4, 1024, 1024, 1024, 16, 64
SCALE = D ** (-0.5)
NCORES = 8
HPC = H // NCORES          # heads per core = 2
HDC = HPC * D              # head-dim columns per core = 128
BT = B * T                 # 4096 tokens
J = MXL + T                # 2048 key positions
P = 128
IR = 512                   # i-range (query block, matmul free dim)
NJT = J // P               # 16 j-tiles
PVW = 2 * D                # per-head lhsT width in P@V: [v (64) | ones (64)]
ET = E // P                # 8 contraction tiles

_CACHE = {}


def _build_program():
    import concourse.mybir as mybir
    import concourse.tile as tile
    from concourse import bacc
    from concourse.masks import make_identity

    fp32 = mybir.dt.float32
    fp16 = mybir.dt.float16
    AF = mybir.ActivationFunctionType

    nc = bacc.Bacc("TRN2", target_bir_lowering=False, debug=False,
                   num_devices=NCORES)

    xTd = nc.dram_tensor("xTd", [E, BT], fp16, kind="ExternalInput")
    wq = nc.dram_tensor("wq", [E, HDC], fp16, kind="ExternalInput")
    wk = nc.dram_tensor("wk", [E, HDC], fp16, kind="ExternalInput")
    wv = nc.dram_tensor("wv", [E, HDC], fp16, kind="ExternalInput")
    wo = nc.dram_tensor("wo", [HDC, E], fp16, kind="ExternalInput")
    bqd = nc.dram_tensor("bqd", [HDC, 1], fp32, kind="ExternalInput")
    bkd = nc.dram_tensor("bkd", [HDC, 1], fp32, kind="ExternalInput")
    bvd = nc.dram_tensor("bvd", [HDC, 1], fp32, kind="ExternalInput")
    kxlT = nc.dram_tensor("kxlT", [HDC, BT], fp16, kind="ExternalInput")
    vxlq = nc.dram_tensor("vxlq", [B, P, MXL // P, HPC * PVW], fp16,
                          kind="ExternalInput")
    erel = nc.dram_tensor("erel", [HPC, J, T], fp16, kind="ExternalInput")
    outp = nc.dram_tensor("outp", [BT, E], fp32, kind="ExternalOutput")
    kvp = nc.dram_tensor("kvp", [BT, 2, HDC], fp32, kind="ExternalOutput")

    with tile.TileContext(nc) as tc:
        with tc.tile_pool(name="const", bufs=1) as constp, \
             tc.tile_pool(name="big", bufs=1) as bigp:
            ident = constp.tile([P, P], fp16)
            make_identity(nc, ident[:])
            bq_sb = constp.tile([HDC, 1], fp32)
            bk_sb = constp.tile([HDC, 1], fp32)
            bv_sb = constp.tile([HDC, 1], fp32)
            wo_sb = constp.tile([HDC, E], fp16)

            kT = [bigp.tile([P, J], fp16, tag=f"kT{b}", name=f"kT{b}")
                  for b in range(B)]
            vpv = [bigp.tile([P, NJT, HPC * PVW], fp16, tag=f"vpv{b}",
                              name=f"vpv{b}") for b in range(B)]
            qT = [bigp.tile([P, T], fp16, tag=f"qT{b}", name=f"qT{b}")
                  for b in range(B)]
            qkvn = [bigp.tile([P, T], fp16, tag=f"qkvn{b}",
                               name=f"qkvn{b}") for b in range(B)]
            er_sb = bigp.tile([P, HPC, NJT, T], fp16)

            # ---- Phase 1: projections --------------------------------------
            with tc.tile_pool(name="w3", bufs=1) as w3p, \
                 tc.tile_pool(name="xt", bufs=2) as xtp, \
                 tc.tile_pool(name="ps1", bufs=2, space="PSUM") as ps1, \
                 tc.tile_pool(name="sb1", bufs=3) as sb1:
                wq_sb = w3p.tile([P, ET, HDC], fp16)
                wk_sb = w3p.tile([P, ET, HDC], fp16)
                wv_sb = w3p.tile([P, ET, HDC], fp16)
                nc.sync.dma_start(
                    wq_sb[:], wq.ap().rearrange("(a p) m -> p a m", p=P))
                nc.sync.dma_start(
                    wk_sb[:], wk.ap().rearrange("(a p) m -> p a m", p=P))
                nc.sync.dma_start(
                    wv_sb[:], wv.ap().rearrange("(a p) m -> p a m", p=P))
                nc.sync.dma_start(bq_sb[:], bqd.ap())
                nc.sync.dma_start(bk_sb[:], bkd.ap())
                nc.sync.dma_start(bv_sb[:], bvd.ap())
                nc.gpsimd.dma_start(wo_sb[:], wo.ap())
                kxl_r = kxlT.ap().rearrange("p (b t) -> p b t", b=B)
                for bb in range(B):
                    nc.gpsimd.dma_start(kT[bb][:, 0:MXL], kxl_r[:, bb, :])
                    nc.gpsimd.dma_start(vpv[bb][:, 0:MXL // P, :],
                                        vxlq.ap()[bb])
                    nc.vector.memset(vpv[bb][:, MXL // P:NJT, D:PVW], 1.0)
                    nc.vector.memset(
                        vpv[bb][:, MXL // P:NJT, PVW + D:2 * PVW], 1.0)
                for h in range(HPC):
                    nc.scalar.dma_start(
                        er_sb[:, h],
                        erel.ap()[h].rearrange("(a p) i -> p a i", p=P))

                for b in range(B):
                    xt = xtp.tile([P, ET, T], fp16)
                    nc.sync.dma_start(
                        xt[:],
                        xTd.ap()[:, b * T:(b + 1) * T]
                           .rearrange("(a p) t -> p a t", p=P))
                    for tr in range(T // IR):
                        ts0 = tr * IR
                        qps = ps1.tile([P, IR], fp32, tag="qps")
                        for e in range(ET):
                            nc.tensor.matmul(qps[:], wq_sb[:, e, :],
                                             xt[:, e, ts0:ts0 + IR],
                                             start=(e == 0),
                                             stop=(e == ET - 1))
                        nc.scalar.activation(
                            qT[b][:, tr * IR:(tr + 1) * IR], qps[:],
                            AF.Identity, bias=bq_sb[:])
                        kps = ps1.tile([P, IR], fp32, tag="kps")
                        for e in range(ET):
                            nc.tensor.matmul(kps[:], wk_sb[:, e, :],
                                             xt[:, e, ts0:ts0 + IR],
                                             start=(e == 0),
                                             stop=(e == ET - 1))
                        nc.scalar.activation(
                            kT[b][:, MXL + tr * IR:MXL + (tr + 1) * IR],
                            kps[:], AF.Identity, bias=bk_sb[:])
                        vps = ps1.tile([P, IR], fp32, tag="vps")
                        for e in range(ET):
                            nc.tensor.matmul(vps[:], wv_sb[:, e, :],
                                             xt[:, e, ts0:ts0 + IR],
                                             start=(e == 0),
                                             stop=(e == ET - 1))
                        vTs = sb1.tile([P, IR], fp16, tag="vTs")
                        nc.scalar.activation(vTs[:], vps[:],
                                             AF.Identity, bias=bv_sb[:])

                        for s in range(IR // P):
                            t0 = tr * IR + s * P
                            jt = MXL // P + t0 // P
                            # v natural via PE transpose; fp32 copy to kv out
                            vtp = ps1.tile([P, P], fp16, tag="tp")
                            nc.tensor.transpose(vtp[:],
                                                vTs[:, s * P:(s + 1) * P],
                                                ident[:])
                            vsb = sb1.tile([P, P], fp32, tag="vsb")
                            nc.scalar.copy(vsb[:], vtp[:])
                            nc.vector.tensor_copy(vpv[b][:, jt, 0:D],
                                                  vsb[:, 0:D])
                            nc.vector.tensor_copy(vpv[b][:, jt, PVW:PVW + D],
                                                  vsb[:, D:2 * D])
                            nc.sync.dma_start(
                                kvp.ap()[b * T + t0:b * T + t0 + P, 1, :],
                                vsb[:])
                            # k natural (current chunk) for the kv output
                            ktp = ps1.tile([P, P], fp16, tag="tp")
                            nc.tensor.transpose(
                                ktp[:],
                                kT[b][:, MXL + t0:MXL + t0 + P],
                                ident[:])
                            ksb = sb1.tile([P, P], fp32, tag="ksb")
                            nc.vector.tensor_copy(ksb[:], ktp[:])
                            nc.sync.dma_start(
                                kvp.ap()[b * T + t0:b * T + t0 + P, 0, :],
                                ksb[:])

            # ---- Phase 2: attention, with interleaved output projection ----
            with tc.tile_pool(name="exs", bufs=3) as exsp, \
                 tc.tile_pool(name="pex", bufs=3) as pexp, \
                 tc.tile_pool(name="sm", bufs=2) as smp, \
                 tc.tile_pool(name="osb", bufs=3) as osbp, \
                 tc.tile_pool(name="psS", bufs=1, space="PSUM") as psS, \
                 tc.tile_pool(name="psPV", bufs=1, space="PSUM") as psPV, \
                 tc.tile_pool(name="psO", bufs=2, space="PSUM") as psO:
                for ir in range(T // IR):
                    i0 = ir * IR
                    nj = (MXL + i0 + IR) // P  # visible j-tiles (12 or 16)
                    for b in range(B):
                        pvps = [psPV.tile([P, IR], fp32, tag=f"pv{h}",
                                          name=f"pv{h}") for h in range(HPC)]
                        for jp in range(nj // 2):
                            sps = [psS.tile([P, 2, IR], fp32, tag=f"s{h}",
                                            name=f"s{h}") for h in range(HPC)]
                            for u in range(2):
                                jt = jp * 2 + u
                                for h in range(HPC):
                                    h0 = h * D
                                    nc.tensor.matmul(
                                        sps[h][:, u, :],
                                        kT[b][h0:h0 + D, jt * P:(jt + 1) * P],
                                        qT[b][h0:h0 + D, i0:i0 + IR],
                                        start=True, stop=True)
                            for h in range(HPC):
                                exs = exsp.tile([P, 2, IR], fp16, tag="exs")
                                nc.scalar.activation(exs[:], sps[h][:],
                                                     AF.Exp, scale=SCALE)
                                pex = pexp.tile([P, 2, IR], fp16, tag="pex")
                                nc.vector.tensor_mul(
                                    pex[:], exs[:],
                                    er_sb[:, h, jp * 2:jp * 2 + 2,
                                          i0:i0 + IR])
                                for u in range(2):
                                    jt = jp * 2 + u
                                    nc.tensor.matmul(
                                        pvps[h][:],
                                        vpv[b][:, jt,
                                               h * PVW:(h + 1) * PVW],
                                        pex[:, u, :],
                                        start=(jt == 0), stop=(jt == nj - 1))
                        for h in range(HPC):
                            h0 = h * D
                            rs = smp.tile([D, IR], fp32, tag="rs")
                            nc.vector.tensor_copy(rs[:], pvps[h][D:2 * D, :])
                            rb = smp.tile([D, IR], fp32, tag="rb")
                            nc.vector.reciprocal_approx_fast(rb[:], rs[:])
                            nc.vector.tensor_mul(
                                qkvn[b][h0:h0 + D, i0:i0 + IR],
                                pvps[h][0:D, :], rb[:])
                        # partial output projection for this (b, i-range)
                        for tt in range(IR // P):
                            t0 = i0 + tt * P
                            for eh in range(E // 512):
                                ops = psO.tile([P, 512], fp32, tag="o")
                                nc.tensor.matmul(
                                    ops[:], qkvn[b][:, t0:t0 + P],
                                    wo_sb[:, eh * 512:(eh + 1) * 512],
                                    start=True, stop=True)
                                osb = osbp.tile([P, 512], fp32, tag="osb")
                                nc.vector.tensor_copy(osb[:], ops[:])
                                nc.sync.dma_start(
                                    outp.ap()[b * T + t0:b * T + t0 + P,
                                              eh * 512:(eh + 1) * 512],
                                    osb[:])

    nc.compile()
    return nc


def _get_program():
    if "nc" not in _CACHE:
        _CACHE["nc"] = _build_program()
    return _CACHE["nc"]


def _prep_inputs(x, xl, rel, Wq, bq, Wk, bk, Wv, bv, Wo):
    """Host-side sharding/layout prep. Returns per-core input maps."""
    f16 = np.float16
    xT = np.ascontiguousarray(x.reshape(BT, E).T).astype(f16)   # [E, BT]

    # mask (j >= i + MXL + 1) and rel bias folded into exp(rel * SCALE)
    jj = np.arange(J, dtype=np.int64)[:, None]
    ii = np.arange(T, dtype=np.int64)[None, :]
    maskT = jj >= (ii + MXL + 1)                                # [J, T]

    in_maps = []
    for c in range(NCORES):
        cs = slice(c * HDC, (c + 1) * HDC)
        relc = np.exp(rel[c * HPC:(c + 1) * HPC].transpose(0, 2, 1) * SCALE)
        relc[:, maskT] = 0.0
        # [B, P, MXL//P, 2*PVW]: per-j-tile rows [vA | 1s | vB | 1s]
        va = xl[:, :, 1, cs].reshape(B, MXL // P, P, HPC, D)
        va = va.transpose(0, 2, 1, 3, 4)             # [B, P, jt, h, D]
        vxlq = np.ones((B, P, MXL // P, HPC * PVW), np.float16)
        vxlq[:, :, :, 0:D] = va[:, :, :, 0]
        vxlq[:, :, :, PVW:PVW + D] = va[:, :, :, 1]
        in_maps.append({
            "xTd": xT,
            "wq": np.ascontiguousarray(Wq[:, cs] * SCALE).astype(f16),
            "wk": np.ascontiguousarray(Wk[:, cs]).astype(f16),
            "wv": np.ascontiguousarray(Wv[:, cs]).astype(f16),
            "wo": np.ascontiguousarray(Wo[cs, :]).astype(f16),
            "bqd": np.ascontiguousarray(
                (bq[cs] * SCALE).reshape(HDC, 1).astype(np.float32)),
            "bkd": np.ascontiguousarray(bk[cs].reshape(HDC, 1)),
            "bvd": np.ascontiguousarray(bv[cs].reshape(HDC, 1)),
            "kxlT": np.ascontiguousarray(
                xl[:, :, 0, cs].reshape(BT, HDC).T).astype(f16),
            "vxlq": vxlq,
            "erel": np.ascontiguousarray(relc).astype(f16),
        })
    return in_maps


def _run(inputs, trace=False, tmpdir=None, trace_cores=None):
    from concourse.bass_utils import run_bass_kernel_spmd

    f = lambda k: np.asarray(inputs[k], np.float32)
    in_maps = _prep_inputs(f("x"), f("xl_memory"), f("relative_positions"),
                           f("Wq"), f("bq"), f("Wk"), f("bk"),
                           f("Wv"), f("bv"), f("Wo"))
    bo = f("bo")

    nc = _get_program()
    kw = {}
    if trace:
        kw.update(trace=True, tmpdir=tmpdir, trace_cores=trace_cores)
    res = run_bass_kernel_spmd(nc, in_maps, list(range(NCORES)), **kw)

    out = np.zeros((BT, E), np.float32)
    kv = np.empty((B, T, 2, H * D), np.float32)
    for c in range(NCORES):
        cs = slice(c * HDC, (c + 1) * HDC)
        out += res.results[c]["outp"]
        kv[:, :, :, cs] = res.results[c]["kvp"].reshape(B, T, 2, HDC)
    out = out.reshape(B, T, E) + bo
    return (out, kv), res


def kernel(**inputs):
    outs, _ = _run(inputs, trace=False)
    return outs


# revision 17
# speedup vs baseline: 1.1036x; 1.1036x over previous
# Trainium2 Bass kernel for nn_CausalSelfAttention_58239756533763.
#
# Sharding: tensor-parallel over heads. 16 heads / 8 cores = 2 heads per
# core. Each core computes q/k/v projections for its 2 heads (all 4
# batches), attention with XL memory, and a partial output projection
# against its 128 rows of Wo. The host sums the 8 partial outputs (the
# "all-reduce" of the sharding hint, done at unshard time) and concatenates
# the per-core k/v slices into the kv_to_add_xl output.
#
# On-chip layout: activations flow in the transposed ("feature on
# partitions") layout so only k/v need on-device 128x128 PE transposes
# (for the kv output / P@V operand):
#   x^T, q^T, k^T, v^T are [feature, token]; scores are S^T = [j, i].
#   Softmax denominators come from a 64-wide ones block appended to v in
#   the P@V matmul, so the sums land on partitions 64:128 of the same
#   PSUM tile (vectorized reciprocal, no cross-partition reduce).
#   exp((S + rel) * s) = exp(S * s) * exp(rel * s): the host precomputes
#   exp(rel^T * s) in fp16 with masked (j, i) entries set to 0, so the
#   bias-add becomes an fp16 multiply and causal masking is exact.
# Matmuls run in fp16 (one PE pass, ~5e-4 quantization) with fp32 PSUM
# accumulation; softmax exp runs on the Scalar engine in fp32.

import numpy as np

B, T, MXL, E, H, D = 4, 1024, 1024, 1024, 16, 64
SCALE = D ** (-0.5)
NCORES = 8
HPC = H // NCORES          # heads per core = 2
HDC = HPC * D              # head-dim columns per core = 128
BT = B * T                 # 4096 tokens
J = MXL + T                # 2048 key positions
P = 128
IR = 512                   # i-range (query block, matmul free dim)
NJT = J // P               # 16 j-tiles
PVW = 2 * D                # per-head lhsT width in P@V: [v (64) | ones (64)]
ET = E // P                # 8 contraction tiles

_CACHE = {}


def _build_program():
    import concourse.mybir as mybir
    import concourse.tile as tile
    from concourse import bacc
    from concourse.masks import make_identity

    fp32 = mybir.dt.float32
    fp16 = mybir.dt.float16
    AF = mybir.ActivationFunctionType

    nc = bacc.Bacc("TRN2", target_bir_lowering=False, debug=False,
                   num_devices=NCORES)

    xTd = nc.dram_tensor("xTd", [E, BT], fp16, kind="ExternalInput")
    wq = nc.dram_tensor("wq", [E, HDC], fp16, kind="ExternalInput")
    wk = nc.dram_tensor("wk", [E, HDC], fp16, kind="ExternalInput")
    wv = nc.dram_tensor("wv", [E, HDC], fp16, kind="ExternalInput")
    wo = nc.dram_tensor("wo", [HDC, E], fp16, kind="ExternalInput")
    bqd = nc.dram_tensor("bqd", [HDC, 1], fp32, kind="ExternalInput")
    bkd = nc.dram_tensor("bkd", [HDC, 1], fp32, kind="ExternalInput")
    bvd = nc.dram_tensor("bvd", [HDC, 1], fp32, kind="ExternalInput")
    kxlT = nc.dram_tensor("kxlT", [HDC, BT], fp16, kind="ExternalInput")
    vxlq = nc.dram_tensor("vxlq", [B, P, MXL // P, HPC * PVW], fp16,
                          kind="ExternalInput")
    erel = nc.dram_tensor("erel", [HPC, J, T], fp16, kind="ExternalInput")
    outp = nc.dram_tensor("outp", [BT, E], fp32, kind="ExternalOutput")
    kvp = nc.dram_tensor("kvp", [BT, 2, HDC], fp32, kind="ExternalOutput")

    with tile.TileContext(nc) as tc:
        with tc.tile_pool(name="const", bufs=1) as constp, \
             tc.tile_pool(name="big", bufs=1) as bigp:
            ident = constp.tile([P, P], fp16)
            make_identity(nc, ident[:])
            bq_sb = constp.tile([HDC, 1], fp32)
            bk_sb = constp.tile([HDC, 1], fp32)
            bv_sb = constp.tile([HDC, 1], fp32)
            wo_sb = constp.tile([HDC, E], fp16)

            kT = [bigp.tile([P, J], fp16, tag=f"kT{b}", name=f"kT{b}")
                  for b in range(B)]
            vpv = [bigp.tile([P, NJT, HPC * PVW], fp16, tag=f"vpv{b}",
                              name=f"vpv{b}") for b in range(B)]
            qT = [bigp.tile([P, T], fp16, tag=f"qT{b}", name=f"qT{b}")
                  for b in range(B)]
            qkvn = [bigp.tile([P, T], fp16, tag=f"qkvn{b}",
                               name=f"qkvn{b}") for b in range(B)]
            er_sb = bigp.tile([P, HPC, NJT, T], fp16)

            # ---- Phase 1: projections --------------------------------------
            with tc.tile_pool(name="w3", bufs=1) as w3p, \
                 tc.tile_pool(name="xt", bufs=2) as xtp, \
                 tc.tile_pool(name="ps1", bufs=2, space="PSUM") as ps1, \
                 tc.tile_pool(name="sb1", bufs=5) as sb1:
                wq_sb = w3p.tile([P, ET, HDC], fp16)
                wk_sb = w3p.tile([P, ET, HDC], fp16)
                wv_sb = w3p.tile([P, ET, HDC], fp16)
                nc.sync.dma_start(
                    wq_sb[:], wq.ap().rearrange("(a p) m -> p a m", p=P))
                nc.sync.dma_start(
                    wk_sb[:], wk.ap().rearrange("(a p) m -> p a m", p=P))
                nc.sync.dma_start(
                    wv_sb[:], wv.ap().rearrange("(a p) m -> p a m", p=P))
                nc.sync.dma_start(bq_sb[:], bqd.ap())
                nc.sync.dma_start(bk_sb[:], bkd.ap())
                nc.sync.dma_start(bv_sb[:], bvd.ap())
                nc.gpsimd.dma_start(wo_sb[:], wo.ap())
                kxl_r = kxlT.ap().rearrange("p (b t) -> p b t", b=B)
                for bb in range(B):
                    nc.gpsimd.dma_start(kT[bb][:, 0:MXL], kxl_r[:, bb, :])
                    nc.gpsimd.dma_start(vpv[bb][:, 0:MXL // P, :],
                                        vxlq.ap()[bb])
                    nc.vector.memset(vpv[bb][:, MXL // P:NJT, D:PVW], 1.0)
                    nc.vector.memset(
                        vpv[bb][:, MXL // P:NJT, PVW + D:2 * PVW], 1.0)
                for ir in range(T // IR):
                    for h in range(HPC):
                        nc.scalar.dma_start(
                            er_sb[:, h, :, ir * IR:(ir + 1) * IR],
                            erel.ap()[h][:, ir * IR:(ir + 1) * IR]
                                .rearrange("(a p) i -> p a i", p=P))

                xts = []
                for b in range(B):
                    xt = xtp.tile([P, ET, T], fp16, tag="xt", name=f"xt{b}")
                    nc.sync.dma_start(
                        xt[:],
                        xTd.ap()[:, b * T:(b + 1) * T]
                           .rearrange("(a p) t -> p a t", p=P))
                    xts.append(xt)
                for b in range(B):
                    xt = xts[b]
                    for tr in range(T // IR):
                        ts0 = tr * IR
                        qps = ps1.tile([P, IR], fp32, tag="qps")
                        for e in range(ET):
                            nc.tensor.matmul(qps[:], wq_sb[:, e, :],
                                             xt[:, e, ts0:ts0 + IR],
                                             start=(e == 0),
                                             stop=(e == ET - 1))
                        nc.scalar.activation(
                            qT[b][:, tr * IR:(tr + 1) * IR], qps[:],
                            AF.Identity, bias=bq_sb[:])
                        kps = ps1.tile([P, IR], fp32, tag="kps")
                        for e in range(ET):
                            nc.tensor.matmul(kps[:], wk_sb[:, e, :],
                                             xt[:, e, ts0:ts0 + IR],
                                             start=(e == 0),
                                             stop=(e == ET - 1))
                        nc.scalar.activation(
                            kT[b][:, MXL + tr * IR:MXL + (tr + 1) * IR],
                            kps[:], AF.Identity, bias=bk_sb[:])
                        vps = ps1.tile([P, IR], fp32, tag="vps")
                        for e in range(ET):
                            nc.tensor.matmul(vps[:], wv_sb[:, e, :],
                                             xt[:, e, ts0:ts0 + IR],
                                             start=(e == 0),
                                             stop=(e == ET - 1))
                        vTs = sb1.tile([P, IR], fp16, tag="vTs")
                        nc.scalar.activation(vTs[:], vps[:],
                                             AF.Identity, bias=bv_sb[:])

                        for s in range(IR // P):
                            t0 = tr * IR + s * P
                            jt = MXL // P + t0 // P
                            # v natural via PE transpose; fp32 copy to kv out
                            vtp = ps1.tile([P, P], fp16, tag="tp")
                            nc.tensor.transpose(vtp[:],
                                                vTs[:, s * P:(s + 1) * P],
                                                ident[:])
                            vsb = sb1.tile([P, P], fp32, tag="vsb")
                            nc.scalar.copy(vsb[:], vtp[:])
                            nc.vector.tensor_copy(vpv[b][:, jt, 0:D],
                                                  vsb[:, 0:D])
                            nc.vector.tensor_copy(vpv[b][:, jt, PVW:PVW + D],
                                                  vsb[:, D:2 * D])
                            nc.gpsimd.dma_start(
                                kvp.ap()[b * T + t0:b * T + t0 + P, 1, :],
                                vsb[:])
                            # k natural (current chunk) for the kv output
                            ktp = ps1.tile([P, P], fp16, tag="tp")
                            nc.tensor.transpose(
                                ktp[:],
                                kT[b][:, MXL + t0:MXL + t0 + P],
                                ident[:])
                            ksb = sb1.tile([P, P], fp32, tag="ksb")
                            nc.vector.tensor_copy(ksb[:], ktp[:])
                            nc.gpsimd.dma_start(
                                kvp.ap()[b * T + t0:b * T + t0 + P, 0, :],
                                ksb[:])

            # ---- Phase 2: attention, with interleaved output projection ----
            with tc.tile_pool(name="exs", bufs=3) as exsp, \
                 tc.tile_pool(name="pex", bufs=3) as pexp, \
                 tc.tile_pool(name="sm", bufs=2) as smp, \
                 tc.tile_pool(name="osb", bufs=3) as osbp, \
                 tc.tile_pool(name="psS", bufs=1, space="PSUM") as psS, \
                 tc.tile_pool(name="psPV", bufs=1, space="PSUM") as psPV, \
                 tc.tile_pool(name="psO", bufs=2, space="PSUM") as psO:
                for ir in range(T // IR):
                    i0 = ir * IR
                    nj = (MXL + i0 + IR) // P  # visible j-tiles (12 or 16)
                    for b in range(B):
                        pvps = [psPV.tile([P, IR], fp32, tag=f"pv{h}",
                                          name=f"pv{h}") for h in range(HPC)]
                        for jp in range(nj // 2):
                            sps = [psS.tile([P, 2, IR], fp32, tag=f"s{h}",
                                            name=f"s{h}") for h in range(HPC)]
                            for u in range(2):
                                jt = jp * 2 + u
                                for h in range(HPC):
                                    h0 = h * D
                                    nc.tensor.matmul(
                                        sps[h][:, u, :],
                                        kT[b][h0:h0 + D, jt * P:(jt + 1) * P],
                                        qT[b][h0:h0 + D, i0:i0 + IR],
                                        start=True, stop=True)
                            for h in range(HPC):
                                exs = exsp.tile([P, 2, IR], fp16, tag="exs")
                                nc.scalar.activation(exs[:], sps[h][:],
                                                     AF.Exp, scale=SCALE)
                                pex = pexp.tile([P, 2, IR], fp16, tag="pex")
                                nc.vector.tensor_mul(
                                    pex[:], exs[:],
                                    er_sb[:, h, jp * 2:jp * 2 + 2,
                                          i0:i0 + IR])
                                for u in range(2):
                                    jt = jp * 2 + u
                                    nc.tensor.matmul(
                                        pvps[h][:],
                                        vpv[b][:, jt,
                                               h * PVW:(h + 1) * PVW],
                                        pex[:, u, :],
                                        start=(jt == 0), stop=(jt == nj - 1))
                        for h in range(HPC):
                            h0 = h * D
                            rs = smp.tile([D, IR], fp32, tag="rs")
                            nc.vector.tensor_copy(rs[:], pvps[h][D:2 * D, :])
                            rb = smp.tile([D, IR], fp32, tag="rb")
                            nc.vector.reciprocal_approx_fast(rb[:], rs[:])
                            nc.vector.tensor_mul(
                                qkvn[b][h0:h0 + D, i0:i0 + IR],
                                pvps[h][0:D, :], rb[:])
                        # partial output projection for this (b, i-range)
                        for tt in range(IR // P):
                            t0 = i0 + tt * P
                            for eh in range(E // 512):
                                ops = psO.tile([P, 512], fp32, tag="o")
                                nc.tensor.matmul(
                                    ops[:], qkvn[b][:, t0:t0 + P],
                                    wo_sb[:, eh * 512:(eh + 1) * 512],
                                    start=True, stop=True)
                                osb = osbp.tile([P, 512], fp32, tag="osb")
                                nc.vector.tensor_copy(osb[:], ops[:])
                                nc.sync.dma_start(
                                    outp.ap()[b * T + t0:b * T + t0 + P,
                                              eh * 512:(eh + 1) * 512],
                                    osb[:])

    nc.compile()
    return nc


def _get_program():
    if "nc" not in _CACHE:
        _CACHE["nc"] = _build_program()
    return _CACHE["nc"]


def _prep_inputs(x, xl, rel, Wq, bq, Wk, bk, Wv, bv, Wo):
    """Host-side sharding/layout prep. Returns per-core input maps."""
    f16 = np.float16
    xT = np.ascontiguousarray(x.reshape(BT, E).T).astype(f16)   # [E, BT]

    # mask (j >= i + MXL + 1) and rel bias folded into exp(rel * SCALE)
    jj = np.arange(J, dtype=np.int64)[:, None]
    ii = np.arange(T, dtype=np.int64)[None, :]
    maskT = jj >= (ii + MXL + 1)                                # [J, T]

    in_maps = []
    for c in range(NCORES):
        cs = slice(c * HDC, (c + 1) * HDC)
        relc = np.exp(rel[c * HPC:(c + 1) * HPC].transpose(0, 2, 1) * SCALE)
        relc[:, maskT] = 0.0
        # [B, P, MXL//P, 2*PVW]: per-j-tile rows [vA | 1s | vB | 1s]
        va = xl[:, :, 1, cs].reshape(B, MXL // P, P, HPC, D)
        va = va.transpose(0, 2, 1, 3, 4)             # [B, P, jt, h, D]
        vxlq = np.ones((B, P, MXL // P, HPC * PVW), np.float16)
        vxlq[:, :, :, 0:D] = va[:, :, :, 0]
        vxlq[:, :, :, PVW:PVW + D] = va[:, :, :, 1]
        in_maps.append({
            "xTd": xT,
            "wq": np.ascontiguousarray(Wq[:, cs] * SCALE).astype(f16),
            "wk": np.ascontiguousarray(Wk[:, cs]).astype(f16),
            "wv": np.ascontiguousarray(Wv[:, cs]).astype(f16),
            "wo": np.ascontiguousarray(Wo[cs, :]).astype(f16),
            "bqd": np.ascontiguousarray(
                (bq[cs] * SCALE).reshape(HDC, 1).astype(np.float32)),
            "bkd": np.ascontiguousarray(bk[cs].reshape(HDC, 1)),
            "bvd": np.ascontiguousarray(bv[cs].reshape(HDC, 1)),
            "kxlT": np.ascontiguousarray(
                xl[:, :, 0, cs].reshape(BT, HDC).T).astype(f16),
            "vxlq": vxlq,
            "erel": np.ascontiguousarray(relc).astype(f16),
        })
    return in_maps


def _run(inputs, trace=False, tmpdir=None, trace_cores=None):
    from concourse.bass_utils import run_bass_kernel_spmd

    f = lambda k: np.asarray(inputs[k], np.float32)
    in_maps = _prep_inputs(f("x"), f("xl_memory"), f("relative_positions"),
                           f("Wq"), f("bq"), f("Wk"), f("bk"),
                           f("Wv"), f("bv"), f("Wo"))
    bo = f("bo")

    nc = _get_program()
    kw = {}
    if trace:
        kw.update(trace=True, tmpdir=tmpdir, trace_cores=trace_cores)
    res = run_bass_kernel_spmd(nc, in_maps, list(range(NCORES)), **kw)

    out = np.zeros((BT, E), np.float32)
    kv = np.empty((B, T, 2, H * D), np.float32)
    for c in range(NCORES):
        cs = slice(c * HDC, (c + 1) * HDC)
        out += res.results[c]["outp"]
        kv[:, :, :, cs] = res.results[c]["kvp"].reshape(B, T, 2, HDC)
    out = out.reshape(B, T, E) + bo
    return (out, kv), res


def kernel(**inputs):
    outs, _ = _run(inputs, trace=False)
    return outs


# revision 19
# speedup vs baseline: 1.1240x; 1.0184x over previous
# Trainium2 Bass kernel for nn_CausalSelfAttention_58239756533763.
#
# Sharding: tensor-parallel over heads. 16 heads / 8 cores = 2 heads per
# core. Each core computes q/k/v projections for its 2 heads (all 4
# batches), attention with XL memory, and a partial output projection
# against its 128 rows of Wo. The host sums the 8 partial outputs (the
# "all-reduce" of the sharding hint, done at unshard time) and concatenates
# the per-core k/v slices into the kv_to_add_xl output.
#
# On-chip layout: activations flow in the transposed ("feature on
# partitions") layout so only k/v need on-device 128x128 PE transposes
# (for the kv output / P@V operand):
#   x^T, q^T, k^T, v^T are [feature, token]; scores are S^T = [j, i].
#   Softmax denominators come from a 64-wide ones block appended to v in
#   the P@V matmul, so the sums land on partitions 64:128 of the same
#   PSUM tile (vectorized reciprocal, no cross-partition reduce).
#   exp((S + rel) * s) = exp(S * s) * exp(rel * s): the host precomputes
#   exp(rel^T * s) in fp16 with masked (j, i) entries set to 0, so the
#   bias-add becomes an fp16 multiply and causal masking is exact.
# Matmuls run in fp16 (one PE pass, ~5e-4 quantization) with fp32 PSUM
# accumulation; softmax exp runs on the Scalar engine in fp32.

import numpy as np

B, T, MXL, E, H, D = 4, 1024, 1024, 1024, 16, 64
SCALE = D ** (-0.5)
NCORES = 8
HPC = H // NCORES          # heads per core = 2
HDC = HPC * D              # head-dim columns per core = 128
BT = B * T                 # 4096 tokens
J = MXL + T                # 2048 key positions
P = 128
IR = 512                   # i-range (query block, matmul free dim)
NJT = J // P               # 16 j-tiles
PVW = 2 * D                # per-head lhsT width in P@V: [v (64) | ones (64)]
ET = E // P                # 8 contraction tiles

_CACHE = {}


def _build_program():
    import concourse.mybir as mybir
    import concourse.tile as tile
    from concourse import bacc
    from concourse.masks import make_identity

    fp32 = mybir.dt.float32
    fp16 = mybir.dt.float16
    AF = mybir.ActivationFunctionType

    nc = bacc.Bacc("TRN2", target_bir_lowering=False, debug=False,
                   num_devices=NCORES)

    xTd = nc.dram_tensor("xTd", [E, BT], fp16, kind="ExternalInput")
    wq = nc.dram_tensor("wq", [E, HDC], fp16, kind="ExternalInput")
    wk = nc.dram_tensor("wk", [E, HDC], fp16, kind="ExternalInput")
    wv = nc.dram_tensor("wv", [E, HDC], fp16, kind="ExternalInput")
    wo = nc.dram_tensor("wo", [HDC, E], fp16, kind="ExternalInput")
    bqd = nc.dram_tensor("bqd", [HDC, 1], fp32, kind="ExternalInput")
    bkd = nc.dram_tensor("bkd", [HDC, 1], fp32, kind="ExternalInput")
    bvd = nc.dram_tensor("bvd", [HDC, 1], fp32, kind="ExternalInput")
    kxlT = nc.dram_tensor("kxlT", [HDC, BT], fp16, kind="ExternalInput")
    vxlq = nc.dram_tensor("vxlq", [B, P, MXL // P, HPC * PVW], fp16,
                          kind="ExternalInput")
    erel = nc.dram_tensor("erel", [HPC, J, T], fp16, kind="ExternalInput")
    outp = nc.dram_tensor("outp", [BT, E], fp32, kind="ExternalOutput")
    kvp = nc.dram_tensor("kvp", [BT, 2, HDC], fp32, kind="ExternalOutput")

    with tile.TileContext(nc) as tc:
        with tc.tile_pool(name="const", bufs=1) as constp, \
             tc.tile_pool(name="big", bufs=1) as bigp, \
             tc.tile_pool(name="w3", bufs=1) as w3p, \
             tc.tile_pool(name="xt", bufs=2) as xtp, \
             tc.tile_pool(name="sb1", bufs=5) as sb1, \
             tc.tile_pool(name="exs", bufs=3) as exsp, \
             tc.tile_pool(name="pex", bufs=3) as pexp, \
             tc.tile_pool(name="sm", bufs=2) as smp, \
             tc.tile_pool(name="osb", bufs=3) as osbp, \
             tc.tile_pool(name="psA", bufs=2, space="PSUM") as psA, \
             tc.tile_pool(name="psPV", bufs=1, space="PSUM") as psPV:
            ident = constp.tile([P, P], fp16)
            make_identity(nc, ident[:])
            bq_sb = constp.tile([HDC, 1], fp32)
            bk_sb = constp.tile([HDC, 1], fp32)
            bv_sb = constp.tile([HDC, 1], fp32)
            wo_sb = constp.tile([HDC, E], fp16)

            kT = [bigp.tile([P, J], fp16, tag=f"kT{b}", name=f"kT{b}")
                  for b in range(B)]
            vpv = [bigp.tile([P, NJT, HPC * PVW], fp16, tag=f"vpv{b}",
                             name=f"vpv{b}") for b in range(B)]
            qT = [bigp.tile([P, T], fp16, tag=f"qT{b}", name=f"qT{b}")
                  for b in range(B)]
            qkvn = [bigp.tile([P, T], fp16, tag=f"qkvn{b}",
                              name=f"qkvn{b}") for b in range(B)]
            er_sb = bigp.tile([P, HPC, NJT, T], fp16)

            wq_sb = w3p.tile([P, ET, HDC], fp16)
            wk_sb = w3p.tile([P, ET, HDC], fp16)
            wv_sb = w3p.tile([P, ET, HDC], fp16)
            nc.sync.dma_start(
                wq_sb[:], wq.ap().rearrange("(a p) m -> p a m", p=P))
            nc.sync.dma_start(
                wk_sb[:], wk.ap().rearrange("(a p) m -> p a m", p=P))
            nc.sync.dma_start(
                wv_sb[:], wv.ap().rearrange("(a p) m -> p a m", p=P))
            nc.sync.dma_start(bq_sb[:], bqd.ap())
            nc.sync.dma_start(bk_sb[:], bkd.ap())
            nc.sync.dma_start(bv_sb[:], bvd.ap())
            xts = []
            for b in range(B):
                xt = xtp.tile([P, ET, T], fp16, tag="xt", name=f"xt{b}")
                nc.sync.dma_start(
                    xt[:],
                    xTd.ap()[:, b * T:(b + 1) * T]
                       .rearrange("(a p) t -> p a t", p=P))
                xts.append(xt)
            for ir in range(T // IR):
                for h in range(HPC):
                    nc.scalar.dma_start(
                        er_sb[:, h, :, ir * IR:(ir + 1) * IR],
                        erel.ap()[h][:, ir * IR:(ir + 1) * IR]
                            .rearrange("(a p) i -> p a i", p=P))
            nc.gpsimd.dma_start(wo_sb[:], wo.ap())
            kxl_r = kxlT.ap().rearrange("p (b t) -> p b t", b=B)
            for bb in range(B):
                nc.gpsimd.dma_start(kT[bb][:, 0:MXL], kxl_r[:, bb, :])
                nc.gpsimd.dma_start(vpv[bb][:, 0:MXL // P, :], vxlq.ap()[bb])
                nc.vector.memset(vpv[bb][:, MXL // P:NJT, D:PVW], 1.0)
                nc.vector.memset(
                    vpv[bb][:, MXL // P:NJT, PVW + D:2 * PVW], 1.0)
            pvps = [psPV.tile([P, IR], fp32, tag=f"pv{h}", name=f"pv{h}")
                    for h in range(HPC)]

            def proj(b):
                xt = xts[b]
                for tr in range(T // IR):
                    ts0 = tr * IR
                    qps = psA.tile([P, IR], fp32, tag="pj", name="qps")
                    for e in range(ET):
                        nc.tensor.matmul(qps[:], wq_sb[:, e, :],
                                         xt[:, e, ts0:ts0 + IR],
                                         start=(e == 0), stop=(e == ET - 1))
                    nc.scalar.activation(
                        qT[b][:, ts0:ts0 + IR], qps[:],
                        AF.Identity, bias=bq_sb[:])
                    kps = psA.tile([P, IR], fp32, tag="pj", name="kps")
                    for e in range(ET):
                        nc.tensor.matmul(kps[:], wk_sb[:, e, :],
                                         xt[:, e, ts0:ts0 + IR],
                                         start=(e == 0), stop=(e == ET - 1))
                    nc.scalar.activation(
                        kT[b][:, MXL + ts0:MXL + ts0 + IR],
                        kps[:], AF.Identity, bias=bk_sb[:])
                    vps = psA.tile([P, IR], fp32, tag="pj", name="vps")
                    for e in range(ET):
                        nc.tensor.matmul(vps[:], wv_sb[:, e, :],
                                         xt[:, e, ts0:ts0 + IR],
                                         start=(e == 0), stop=(e == ET - 1))
                    vTs = sb1.tile([P, IR], fp16, tag="vTs")
                    nc.scalar.activation(vTs[:], vps[:],
                                         AF.Identity, bias=bv_sb[:])
                    for s in range(IR // P):
                        t0 = ts0 + s * P
                        jt = MXL // P + t0 // P
                        vtp = psA.tile([P, P], fp16, tag="pj", name="vtp")
                        nc.tensor.transpose(vtp[:], vTs[:, s * P:(s + 1) * P],
                                            ident[:])
                        vsb = sb1.tile([P, P], fp32, tag="vsb")
                        nc.scalar.copy(vsb[:], vtp[:])
                        nc.vector.tensor_copy(vpv[b][:, jt, 0:D], vsb[:, 0:D])
                        nc.vector.tensor_copy(vpv[b][:, jt, PVW:PVW + D],
                                              vsb[:, D:2 * D])
                        nc.gpsimd.dma_start(
                            kvp.ap()[b * T + t0:b * T + t0 + P, 1, :], vsb[:])
                        ktp = psA.tile([P, P], fp16, tag="pj", name="ktp")
                        nc.tensor.transpose(
                            ktp[:], kT[b][:, MXL + t0:MXL + t0 + P], ident[:])
                        ksb = sb1.tile([P, P], fp32, tag="ksb")
                        nc.vector.tensor_copy(ksb[:], ktp[:])
                        nc.gpsimd.dma_start(
                            kvp.ap()[b * T + t0:b * T + t0 + P, 0, :], ksb[:])

            def attn(b):
                for ir in range(T // IR):
                    i0 = ir * IR
                    nj = (MXL + i0 + IR) // P
                    for jp in range(nj // 2):
                        sps = [psA.tile([P, 2, IR], fp32, tag=f"s{h}",
                                        name=f"s{h}", bufs=1)
                               for h in range(HPC)]
                        for u in range(2):
                            jt = jp * 2 + u
                            for h in range(HPC):
                                h0 = h * D
                                nc.tensor.matmul(
                                    sps[h][:, u, :],
                                    kT[b][h0:h0 + D, jt * P:(jt + 1) * P],
                                    qT[b][h0:h0 + D, i0:i0 + IR],
                                    start=True, stop=True)
                        for h in range(HPC):
                            exs = exsp.tile([P, 2, IR], fp16, tag="exs")
                            nc.scalar.activation(exs[:], sps[h][:], AF.Exp,
                                                 scale=SCALE)
                            pex = pexp.tile([P, 2, IR], fp16, tag="pex")
                            nc.vector.tensor_mul(
                                pex[:], exs[:],
                                er_sb[:, h, jp * 2:jp * 2 + 2, i0:i0 + IR])
                            for u in range(2):
                                jt = jp * 2 + u
                                nc.tensor.matmul(
                                    pvps[h][:],
                                    vpv[b][:, jt, h * PVW:(h + 1) * PVW],
                                    pex[:, u, :],
                                    start=(jt == 0), stop=(jt == nj - 1))
                    for h in range(HPC):
                        h0 = h * D
                        rs = smp.tile([D, IR], fp32, tag="rs")
                        nc.vector.tensor_copy(rs[:], pvps[h][D:2 * D, :])
                        rb = smp.tile([D, IR], fp32, tag="rb")
                        nc.vector.reciprocal_approx_fast(rb[:], rs[:])
                        nc.vector.tensor_mul(
                            qkvn[b][h0:h0 + D, i0:i0 + IR],
                            pvps[h][0:D, :], rb[:])
                    for tt in range(IR // P):
                        t0 = i0 + tt * P
                        for eh in range(E // 512):
                            ops = psA.tile([P, 512], fp32, tag="pj",
                                           name="ops")
                            nc.tensor.matmul(
                                ops[:], qkvn[b][:, t0:t0 + P],
                                wo_sb[:, eh * 512:(eh + 1) * 512],
                                start=True, stop=True)
                            osb = osbp.tile([P, 512], fp32, tag="osb")
                            nc.vector.tensor_copy(osb[:], ops[:])
                            nc.sync.dma_start(
                                outp.ap()[b * T + t0:b * T + t0 + P,
                                          eh * 512:(eh + 1) * 512],
                                osb[:])

            proj(0)
            proj(1)
            attn(0)
            proj(2)
            attn(1)
            proj(3)
            attn(2)
            attn(3)

    nc.compile()
    return nc


def _get_program():
    if "nc" not in _CACHE:
        _CACHE["nc"] = _build_program()
    return _CACHE["nc"]


def _prep_inputs(x, xl, rel, Wq, bq, Wk, bk, Wv, bv, Wo):
    """Host-side sharding/layout prep. Returns per-core input maps."""
    f16 = np.float16
    xT = np.ascontiguousarray(x.reshape(BT, E).T).astype(f16)   # [E, BT]

    # mask (j >= i + MXL + 1) and rel bias folded into exp(rel * SCALE)
    jj = np.arange(J, dtype=np.int64)[:, None]
    ii = np.arange(T, dtype=np.int64)[None, :]
    maskT = jj >= (ii + MXL + 1)                                # [J, T]

    in_maps = []
    for c in range(NCORES):
        cs = slice(c * HDC, (c + 1) * HDC)
        relc = np.exp(rel[c * HPC:(c + 1) * HPC].transpose(0, 2, 1) * SCALE)
        relc[:, maskT] = 0.0
        # [B, P, MXL//P, 2*PVW]: per-j-tile rows [vA | 1s | vB | 1s]
        va = xl[:, :, 1, cs].reshape(B, MXL // P, P, HPC, D)
        va = va.transpose(0, 2, 1, 3, 4)             # [B, P, jt, h, D]
        vxlq = np.ones((B, P, MXL // P, HPC * PVW), np.float16)
        vxlq[:, :, :, 0:D] = va[:, :, :, 0]
        vxlq[:, :, :, PVW:PVW + D] = va[:, :, :, 1]
        in_maps.append({
            "xTd": xT,
            "wq": np.ascontiguousarray(Wq[:, cs] * SCALE).astype(f16),
            "wk": np.ascontiguousarray(Wk[:, cs]).astype(f16),
            "wv": np.ascontiguousarray(Wv[:, cs]).astype(f16),
            "wo": np.ascontiguousarray(Wo[cs, :]).astype(f16),
            "bqd": np.ascontiguousarray(
                (bq[cs] * SCALE).reshape(HDC, 1).astype(np.float32)),
            "bkd": np.ascontiguousarray(bk[cs].reshape(HDC, 1)),
            "bvd": np.ascontiguousarray(bv[cs].reshape(HDC, 1)),
            "kxlT": np.ascontiguousarray(
                xl[:, :, 0, cs].reshape(BT, HDC).T).astype(f16),
            "vxlq": vxlq,
            "erel": np.ascontiguousarray(relc).astype(f16),
        })
    return in_maps


def _run(inputs, trace=False, tmpdir=None, trace_cores=None):
    from concourse.bass_utils import run_bass_kernel_spmd

    f = lambda k: np.asarray(inputs[k], np.float32)
    in_maps = _prep_inputs(f("x"), f("xl_memory"), f("relative_positions"),
                           f("Wq"), f("bq"), f("Wk"), f("bk"),
                           f("Wv"), f("bv"), f("Wo"))
    bo = f("bo")

    nc = _get_program()
    kw = {}
    if trace:
        kw.update(trace=True, tmpdir=tmpdir, trace_cores=trace_cores)
    res = run_bass_kernel_spmd(nc, in_maps, list(range(NCORES)), **kw)

    out = np.zeros((BT, E), np.float32)
    kv = np.empty((B, T, 2, H * D), np.float32)
    for c in range(NCORES):
        cs = slice(c * HDC, (c + 1) * HDC)
        out += res.results[c]["outp"]
        kv[:, :, :, cs] = res.results[c]["kvp"].reshape(B, T, 2, HDC)
    out = out.reshape(B, T, E) + bo
    return (out, kv), res


def kernel(**inputs):
    outs, _ = _run(inputs, trace=False)
    return outs


# revision 22
# speedup vs baseline: 1.1639x; 1.0355x over previous
# Trainium2 Bass kernel for nn_CausalSelfAttention_58239756533763.
#
# Sharding: tensor-parallel over heads. 16 heads / 8 cores = 2 heads per
# core. Each core computes q/k/v projections for its 2 heads (all 4
# batches), attention with XL memory, and a partial output projection
# against its 128 rows of Wo. The host sums the 8 partial outputs (the
# "all-reduce" of the sharding hint, done at unshard time) and concatenates
# the per-core k/v slices into the kv_to_add_xl output.
#
# On-chip layout: activations flow in the transposed ("feature on
# partitions") layout so only k/v need on-device 128x128 PE transposes
# (for the kv output / P@V operand):
#   x^T, q^T, k^T, v^T are [feature, token]; scores are S^T = [j, i].
#   Softmax denominators come from a 64-wide ones block appended to v in
#   the P@V matmul, so the sums land on partitions 64:128 of the same
#   PSUM tile (vectorized reciprocal, no cross-partition reduce).
#   exp((S + rel) * s) = exp(S * s) * exp(rel * s): the host precomputes
#   exp(rel^T * s) in fp16 with masked (j, i) entries set to 0, so the
#   bias-add becomes an fp16 multiply and causal masking is exact.
# Matmuls run in fp16 (one PE pass, ~5e-4 quantization) with fp32 PSUM
# accumulation; softmax exp runs on the Scalar engine in fp32.

import numpy as np

B, T, MXL, E, H, D = 4, 1024, 1024, 1024, 16, 64
SCALE = D ** (-0.5)
NCORES = 8
HPC = H // NCORES          # heads per core = 2
HDC = HPC * D              # head-dim columns per core = 128
BT = B * T                 # 4096 tokens
J = MXL + T                # 2048 key positions
P = 128
IR = 512                   # i-range (query block, matmul free dim)
NJT = J // P               # 16 j-tiles
PVW = 2 * D                # per-head lhsT width in P@V: [v (64) | ones (64)]
ET = E // P                # 8 contraction tiles

_CACHE = {}


def _build_program():
    import concourse.mybir as mybir
    import concourse.tile as tile
    from concourse import bacc
    from concourse.masks import make_identity

    fp32 = mybir.dt.float32
    fp16 = mybir.dt.float16
    AF = mybir.ActivationFunctionType

    nc = bacc.Bacc("TRN2", target_bir_lowering=False, debug=False,
                   num_devices=NCORES)

    xTd = nc.dram_tensor("xTd", [E, BT], fp16, kind="ExternalInput")
    wq = nc.dram_tensor("wq", [E, HDC], fp16, kind="ExternalInput")
    wk = nc.dram_tensor("wk", [E, HDC], fp16, kind="ExternalInput")
    wv = nc.dram_tensor("wv", [E, HDC], fp16, kind="ExternalInput")
    wo = nc.dram_tensor("wo", [HDC, E], fp16, kind="ExternalInput")
    bqd = nc.dram_tensor("bqd", [HDC, 1], fp32, kind="ExternalInput")
    bkd = nc.dram_tensor("bkd", [HDC, 1], fp32, kind="ExternalInput")
    bvd = nc.dram_tensor("bvd", [HDC, 1], fp32, kind="ExternalInput")
    kxlT = nc.dram_tensor("kxlT", [HDC, BT], fp16, kind="ExternalInput")
    vxlq = nc.dram_tensor("vxlq", [B, P, MXL // P, HPC * PVW], fp16,
                          kind="ExternalInput")
    erel = nc.dram_tensor("erel", [HPC, J, T], fp16, kind="ExternalInput")
    outp = nc.dram_tensor("outp", [BT, E], fp32, kind="ExternalOutput")
    kvp = nc.dram_tensor("kvp", [BT, 2, HDC], fp32, kind="ExternalOutput")

    with tile.TileContext(nc) as tc:
        with tc.tile_pool(name="const", bufs=1) as constp, \
             tc.tile_pool(name="big", bufs=1) as bigp, \
             tc.tile_pool(name="w3", bufs=1) as w3p, \
             tc.tile_pool(name="xt", bufs=2) as xtp, \
             tc.tile_pool(name="sb1", bufs=4) as sb1, \
             tc.tile_pool(name="exs", bufs=4) as exsp, \
             tc.tile_pool(name="pex", bufs=4) as pexp, \
             tc.tile_pool(name="sm", bufs=2) as smp, \
             tc.tile_pool(name="osb", bufs=2) as osbp, \
             tc.tile_pool(name="psA", bufs=2, space="PSUM") as psA, \
             tc.tile_pool(name="psPV", bufs=1, space="PSUM") as psPV:
            ident = constp.tile([P, P], fp16)
            make_identity(nc, ident[:])
            bq_sb = constp.tile([HDC, 1], fp32)
            bk_sb = constp.tile([HDC, 1], fp32)
            bv_sb = constp.tile([HDC, 1], fp32)
            wo_sb = constp.tile([HDC, E], fp16)

            kT = [bigp.tile([P, J], fp16, tag=f"kT{b}", name=f"kT{b}")
                  for b in range(B)]
            vpv = [bigp.tile([P, NJT, HPC * PVW], fp16, tag=f"vpv{b}",
                             name=f"vpv{b}") for b in range(B)]
            qT = [bigp.tile([P, T], fp16, tag=f"qT{b}", name=f"qT{b}")
                  for b in range(B)]
            qkvn = [bigp.tile([P, T], fp16, tag=f"qkvn{b}",
                              name=f"qkvn{b}") for b in range(B)]
            er_sb = bigp.tile([P, HPC, NJT, T], fp16)

            wq_sb = w3p.tile([P, ET, HDC], fp16)
            wk_sb = w3p.tile([P, ET, HDC], fp16)
            wv_sb = w3p.tile([P, ET, HDC], fp16)
            nc.sync.dma_start(
                wq_sb[:], wq.ap().rearrange("(a p) m -> p a m", p=P))
            nc.sync.dma_start(
                wk_sb[:], wk.ap().rearrange("(a p) m -> p a m", p=P))
            nc.sync.dma_start(
                wv_sb[:], wv.ap().rearrange("(a p) m -> p a m", p=P))
            nc.sync.dma_start(bq_sb[:], bqd.ap())
            nc.sync.dma_start(bk_sb[:], bkd.ap())
            nc.sync.dma_start(bv_sb[:], bvd.ap())
            xts = []
            for b in range(B):
                xt = xtp.tile([P, ET, T], fp16, tag="xt", name=f"xt{b}")
                nc.sync.dma_start(
                    xt[:],
                    xTd.ap()[:, b * T:(b + 1) * T]
                       .rearrange("(a p) t -> p a t", p=P))
                xts.append(xt)
            def er_load(ir):
                for h in range(HPC):
                    nc.scalar.dma_start(
                        er_sb[:, h, :, ir * IR:(ir + 1) * IR],
                        erel.ap()[h][:, ir * IR:(ir + 1) * IR]
                            .rearrange("(a p) i -> p a i", p=P))
            er_load(0)
            nc.gpsimd.dma_start(wo_sb[:], wo.ap())
            kxl_r = kxlT.ap().rearrange("p (b t) -> p b t", b=B)
            for bb in range(B):
                nc.gpsimd.dma_start(kT[bb][:, 0:MXL], kxl_r[:, bb, :])
                nc.gpsimd.dma_start(vpv[bb][:, 0:MXL // P, :], vxlq.ap()[bb])
                nc.vector.memset(vpv[bb][:, MXL // P:NJT, D:PVW], 1.0)
                nc.vector.memset(
                    vpv[bb][:, MXL // P:NJT, PVW + D:2 * PVW], 1.0)

            def proj(b):
                xt = xts[b]
                for tr in range(T // IR):
                    ts0 = tr * IR
                    qps = psA.tile([P, IR], fp32, tag="pj", name="qps")
                    for e in range(ET):
                        nc.tensor.matmul(qps[:], wq_sb[:, e, :],
                                         xt[:, e, ts0:ts0 + IR],
                                         start=(e == 0), stop=(e == ET - 1))
                    nc.scalar.activation(
                        qT[b][:, ts0:ts0 + IR], qps[:],
                        AF.Identity, bias=bq_sb[:])
                    kps = psA.tile([P, IR], fp32, tag="pj", name="kps")
                    for e in range(ET):
                        nc.tensor.matmul(kps[:], wk_sb[:, e, :],
                                         xt[:, e, ts0:ts0 + IR],
                                         start=(e == 0), stop=(e == ET - 1))
                    nc.scalar.activation(
                        kT[b][:, MXL + ts0:MXL + ts0 + IR],
                        kps[:], AF.Identity, bias=bk_sb[:])
                    vps = psA.tile([P, IR], fp32, tag="pj", name="vps")
                    for e in range(ET):
                        nc.tensor.matmul(vps[:], wv_sb[:, e, :],
                                         xt[:, e, ts0:ts0 + IR],
                                         start=(e == 0), stop=(e == ET - 1))
                    vTs = sb1.tile([P, IR], fp16, tag="vTs")
                    nc.scalar.activation(vTs[:], vps[:],
                                         AF.Identity, bias=bv_sb[:])
                    for s in range(IR // P):
                        t0 = ts0 + s * P
                        jt = MXL // P + t0 // P
                        vtp = psA.tile([P, P], fp16, tag="pj", name="vtp")
                        nc.tensor.transpose(vtp[:], vTs[:, s * P:(s + 1) * P],
                                            ident[:])
                        vsb = sb1.tile([P, P], fp32, tag="vsb")
                        nc.scalar.copy(vsb[:], vtp[:])
                        nc.vector.tensor_copy(vpv[b][:, jt, 0:D], vsb[:, 0:D])
                        nc.vector.tensor_copy(vpv[b][:, jt, PVW:PVW + D],
                                              vsb[:, D:2 * D])
                        nc.gpsimd.dma_start(
                            kvp.ap()[b * T + t0:b * T + t0 + P, 1, :], vsb[:])
                        ktp = psA.tile([P, P], fp16, tag="pj", name="ktp")
                        nc.tensor.transpose(
                            ktp[:], kT[b][:, MXL + t0:MXL + t0 + P], ident[:])
                        ksb = sb1.tile([P, P], fp32, tag="ksb")
                        nc.vector.tensor_copy(ksb[:], ktp[:])
                        nc.gpsimd.dma_start(
                            kvp.ap()[b * T + t0:b * T + t0 + P, 0, :], ksb[:])

            def attn(b):
                for ir in range(T // IR):
                    i0 = ir * IR
                    nj = (MXL + i0 + IR) // P
                    pvps = [psPV.tile([P, IR], fp32, tag=f"pv{h}",
                                      name=f"pv{h}", bufs=1)
                            for h in range(HPC)]
                    for jp in range(nj // 2):
                        sps = [psA.tile([P, 2, IR], fp32, tag=f"s{h}",
                                        name=f"s{h}", bufs=1)
                               for h in range(HPC)]
                        for u in range(2):
                            jt = jp * 2 + u
                            for h in range(HPC):
                                h0 = h * D
                                nc.tensor.matmul(
                                    sps[h][:, u, :],
                                    kT[b][h0:h0 + D, jt * P:(jt + 1) * P],
                                    qT[b][h0:h0 + D, i0:i0 + IR],
                                    start=True, stop=True)
                        for h in range(HPC):
                            exs = exsp.tile([P, 2, IR], fp16, tag="exs")
                            nc.scalar.activation(exs[:], sps[h][:], AF.Exp,
                                                 scale=SCALE)
                            pex = pexp.tile([P, 2, IR], fp16, tag="pex")
                            nc.vector.tensor_mul(
                                pex[:], exs[:],
                                er_sb[:, h, jp * 2:jp * 2 + 2, i0:i0 + IR])
                            for u in range(2):
                                jt = jp * 2 + u
                                nc.tensor.matmul(
                                    pvps[h][:],
                                    vpv[b][:, jt, h * PVW:(h + 1) * PVW],
                                    pex[:, u, :],
                                    start=(jt == 0), stop=(jt == nj - 1))
                    for h in range(HPC):
                        h0 = h * D
                        rs = smp.tile([D, IR], fp32, tag="rs")
                        nc.vector.tensor_copy(rs[:], pvps[h][D:2 * D, :])
                        rb = smp.tile([D, IR], fp32, tag="rb")
                        nc.vector.reciprocal_approx_fast(rb[:], rs[:])
                        nc.vector.tensor_mul(
                            qkvn[b][h0:h0 + D, i0:i0 + IR],
                            pvps[h][0:D, :], rb[:])
                    for tt in range(IR // P):
                        t0 = i0 + tt * P
                        for eh in range(E // 512):
                            ops = psA.tile([P, 512], fp32, tag="pj",
                                           name="ops")
                            nc.tensor.matmul(
                                ops[:], qkvn[b][:, t0:t0 + P],
                                wo_sb[:, eh * 512:(eh + 1) * 512],
                                start=True, stop=True)
                            osb = osbp.tile([P, 512], fp32, tag="osb")
                            nc.vector.tensor_copy(osb[:], ops[:])
                            nc.sync.dma_start(
                                outp.ap()[b * T + t0:b * T + t0 + P,
                                          eh * 512:(eh + 1) * 512],
                                osb[:])

            proj(0)
            proj(1)
            er_load(1)
            attn(0)
            proj(2)
            attn(1)
            proj(3)
            attn(2)
            attn(3)

    nc.compile()
    return nc


def _get_program():
    if "nc" not in _CACHE:
        _CACHE["nc"] = _build_program()
    return _CACHE["nc"]


def _prep_inputs(x, xl, rel, Wq, bq, Wk, bk, Wv, bv, Wo):
    """Host-side sharding/layout prep. Returns per-core input maps."""
    f16 = np.float16
    xT = np.ascontiguousarray(x.reshape(BT, E).T).astype(f16)   # [E, BT]

    # mask (j >= i + MXL + 1) and rel bias folded into exp(rel * SCALE)
    jj = np.arange(J, dtype=np.int64)[:, None]
    ii = np.arange(T, dtype=np.int64)[None, :]
    maskT = jj >= (ii + MXL + 1)                                # [J, T]

    in_maps = []
    for c in range(NCORES):
        cs = slice(c * HDC, (c + 1) * HDC)
        relc = np.exp(rel[c * HPC:(c + 1) * HPC].transpose(0, 2, 1) * SCALE)
        relc[:, maskT] = 0.0
        # [B, P, MXL//P, 2*PVW]: per-j-tile rows [vA | 1s | vB | 1s]
        va = xl[:, :, 1, cs].reshape(B, MXL // P, P, HPC, D)
        va = va.transpose(0, 2, 1, 3, 4)             # [B, P, jt, h, D]
        vxlq = np.ones((B, P, MXL // P, HPC * PVW), np.float16)
        vxlq[:, :, :, 0:D] = va[:, :, :, 0]
        vxlq[:, :, :, PVW:PVW + D] = va[:, :, :, 1]
        in_maps.append({
            "xTd": xT,
            "wq": np.ascontiguousarray(Wq[:, cs] * SCALE).astype(f16),
            "wk": np.ascontiguousarray(Wk[:, cs]).astype(f16),
            "wv": np.ascontiguousarray(Wv[:, cs]).astype(f16),
            "wo": np.ascontiguousarray(Wo[cs, :]).astype(f16),
            "bqd": np.ascontiguousarray(
                (bq[cs] * SCALE).reshape(HDC, 1).astype(np.float32)),
            "bkd": np.ascontiguousarray(bk[cs].reshape(HDC, 1)),
            "bvd": np.ascontiguousarray(bv[cs].reshape(HDC, 1)),
            "kxlT": np.ascontiguousarray(
                xl[:, :, 0, cs].reshape(BT, HDC).T).astype(f16),
            "vxlq": vxlq,
            "erel": np.ascontiguousarray(relc).astype(f16),
        })
    return in_maps


def _run(inputs, trace=False, tmpdir=None, trace_cores=None):
    from concourse.bass_utils import run_bass_kernel_spmd

    f = lambda k: np.asarray(inputs[k], np.float32)
    in_maps = _prep_inputs(f("x"), f("xl_memory"), f("relative_positions"),
                           f("Wq"), f("bq"), f("Wk"), f("bk"),
                           f("Wv"), f("bv"), f("Wo"))
    bo = f("bo")

    nc = _get_program()
    kw = {}
    if trace:
        kw.update(trace=True, tmpdir=tmpdir, trace_cores=trace_cores)
    res = run_bass_kernel_spmd(nc, in_maps, list(range(NCORES)), **kw)

    out = np.zeros((BT, E), np.float32)
    kv = np.empty((B, T, 2, H * D), np.float32)
    for c in range(NCORES):
        cs = slice(c * HDC, (c + 1) * HDC)
        out += res.results[c]["outp"]
        kv[:, :, :, cs] = res.results[c]["kvp"].reshape(B, T, 2, HDC)
    out = out.reshape(B, T, E) + bo
    return (out, kv), res


def kernel(**inputs):
    outs, _ = _run(inputs, trace=False)
    return outs


# revision 23
# speedup vs baseline: 1.1861x; 1.0191x over previous
# Trainium2 Bass kernel for nn_CausalSelfAttention_58239756533763.
#
# Sharding: tensor-parallel over heads. 16 heads / 8 cores = 2 heads per
# core. Each core computes q/k/v projections for its 2 heads (all 4
# batches), attention with XL memory, and a partial output projection
# against its 128 rows of Wo. The host sums the 8 partial outputs (the
# "all-reduce" of the sharding hint, done at unshard time) and concatenates
# the per-core k/v slices into the kv_to_add_xl output.
#
# On-chip layout: activations flow in the transposed ("feature on
# partitions") layout so only k/v need on-device 128x128 PE transposes
# (for the kv output / P@V operand):
#   x^T, q^T, k^T, v^T are [feature, token]; scores are S^T = [j, i].
#   Softmax denominators come from a 64-wide ones block appended to v in
#   the P@V matmul, so the sums land on partitions 64:128 of the same
#   PSUM tile (vectorized reciprocal, no cross-partition reduce).
#   exp((S + rel) * s) = exp(S * s) * exp(rel * s): the host precomputes
#   exp(rel^T * s) in fp16 with masked (j, i) entries set to 0, so the
#   bias-add becomes an fp16 multiply and causal masking is exact.
# Matmuls run in fp16 (one PE pass, ~5e-4 quantization) with fp32 PSUM
# accumulation; softmax exp runs on the Scalar engine in fp32.

import numpy as np

B, T, MXL, E, H, D = 4, 1024, 1024, 1024, 16, 64
SCALE = D ** (-0.5)
NCORES = 8
HPC = H // NCORES          # heads per core = 2
HDC = HPC * D              # head-dim columns per core = 128
BT = B * T                 # 4096 tokens
J = MXL + T                # 2048 key positions
P = 128
IR = 512                   # i-range (query block, matmul free dim)
NJT = J // P               # 16 j-tiles
PVW = 2 * D                # per-head lhsT width in P@V: [v (64) | ones (64)]
ET = E // P                # 8 contraction tiles

_CACHE = {}


def _build_program():
    import concourse.mybir as mybir
    import concourse.tile as tile
    from concourse import bacc
    from concourse.masks import make_identity

    fp32 = mybir.dt.float32
    fp16 = mybir.dt.float16
    AF = mybir.ActivationFunctionType

    nc = bacc.Bacc("TRN2", target_bir_lowering=False, debug=False,
                   num_devices=NCORES)

    xTd = nc.dram_tensor("xTd", [E, BT], fp16, kind="ExternalInput")
    wq = nc.dram_tensor("wq", [E, HDC], fp16, kind="ExternalInput")
    wk = nc.dram_tensor("wk", [E, HDC], fp16, kind="ExternalInput")
    wv = nc.dram_tensor("wv", [E, HDC], fp16, kind="ExternalInput")
    wo = nc.dram_tensor("wo", [HDC, E], fp16, kind="ExternalInput")
    bqd = nc.dram_tensor("bqd", [HDC, 1], fp32, kind="ExternalInput")
    bkd = nc.dram_tensor("bkd", [HDC, 1], fp32, kind="ExternalInput")
    bvd = nc.dram_tensor("bvd", [HDC, 1], fp32, kind="ExternalInput")
    kxlT = nc.dram_tensor("kxlT", [HDC, BT], fp16, kind="ExternalInput")
    vxlq = nc.dram_tensor("vxlq", [B, P, MXL // P, HPC * PVW], fp16,
                          kind="ExternalInput")
    erel = nc.dram_tensor("erel", [HPC, J, T], fp16, kind="ExternalInput")
    outp = nc.dram_tensor("outp", [BT, E], fp32, kind="ExternalOutput")
    kvp = nc.dram_tensor("kvp", [BT, 2, HDC], fp32, kind="ExternalOutput")

    with tile.TileContext(nc) as tc:
        with tc.tile_pool(name="const", bufs=1) as constp, \
             tc.tile_pool(name="big", bufs=1) as bigp, \
             tc.tile_pool(name="w3", bufs=1) as w3p, \
             tc.tile_pool(name="xt", bufs=3) as xtp, \
             tc.tile_pool(name="sb1", bufs=4) as sb1, \
             tc.tile_pool(name="exs", bufs=4) as exsp, \
             tc.tile_pool(name="pex", bufs=4) as pexp, \
             tc.tile_pool(name="sm", bufs=3) as smp, \
             tc.tile_pool(name="osb", bufs=3) as osbp, \
             tc.tile_pool(name="psA", bufs=2, space="PSUM") as psA, \
             tc.tile_pool(name="psPV", bufs=1, space="PSUM") as psPV:
            ident = constp.tile([P, P], fp16)
            make_identity(nc, ident[:])
            bq_sb = constp.tile([HDC, 1], fp32)
            bk_sb = constp.tile([HDC, 1], fp32)
            bv_sb = constp.tile([HDC, 1], fp32)
            wo_sb = constp.tile([HDC, E], fp16)

            kT = [bigp.tile([P, J], fp16, tag=f"kT{b}", name=f"kT{b}")
                  for b in range(B)]
            vpv = [bigp.tile([P, NJT, HPC * PVW], fp16, tag=f"vpv{b}",
                             name=f"vpv{b}") for b in range(B)]
            qT = [bigp.tile([P, T], fp16, tag=f"qT{b}", name=f"qT{b}")
                  for b in range(B)]
            qkvn = [bigp.tile([P, T], fp16, tag=f"qkvn{b}",
                              name=f"qkvn{b}") for b in range(B)]
            er_sb = bigp.tile([P, HPC, NJT, T], fp16)

            wq_sb = w3p.tile([P, ET, HDC], fp16)
            wk_sb = w3p.tile([P, ET, HDC], fp16)
            wv_sb = w3p.tile([P, ET, HDC], fp16)
            nc.sync.dma_start(
                wq_sb[:], wq.ap().rearrange("(a p) m -> p a m", p=P))
            nc.sync.dma_start(
                wk_sb[:], wk.ap().rearrange("(a p) m -> p a m", p=P))
            nc.sync.dma_start(
                wv_sb[:], wv.ap().rearrange("(a p) m -> p a m", p=P))
            nc.sync.dma_start(bq_sb[:], bqd.ap())
            nc.sync.dma_start(bk_sb[:], bkd.ap())
            nc.sync.dma_start(bv_sb[:], bvd.ap())
            xts = {}
            def xt_load(b, tr):
                xt = xtp.tile([P, ET, IR], fp16, tag="xt",
                              name=f"xt{b}_{tr}")
                col0 = b * T + tr * IR
                nc.sync.dma_start(
                    xt[:],
                    xTd.ap()[:, col0:col0 + IR]
                       .rearrange("(a p) t -> p a t", p=P))
                xts[(b, tr)] = xt
            xt_load(0, 0)
            xt_load(0, 1)
            def er_load(ir):
                for h in range(HPC):
                    nc.scalar.dma_start(
                        er_sb[:, h, :, ir * IR:(ir + 1) * IR],
                        erel.ap()[h][:, ir * IR:(ir + 1) * IR]
                            .rearrange("(a p) i -> p a i", p=P))
            er_load(0)
            def feeds():
                nc.gpsimd.dma_start(wo_sb[:], wo.ap())
                kxl_r = kxlT.ap().rearrange("p (b t) -> p b t", b=B)
                for bb in range(B):
                    nc.gpsimd.dma_start(kT[bb][:, 0:MXL], kxl_r[:, bb, :])
                    nc.gpsimd.dma_start(vpv[bb][:, 0:MXL // P, :],
                                        vxlq.ap()[bb])
                    nc.vector.memset(vpv[bb][:, MXL // P:NJT, D:PVW], 1.0)
                    nc.vector.memset(
                        vpv[bb][:, MXL // P:NJT, PVW + D:2 * PVW], 1.0)

            def proj(b):
                for tr in range(T // IR):
                    nxt = (b, tr + 1) if tr + 1 < T // IR else (b + 1, 0)
                    if nxt[0] < B and nxt not in xts:
                        xt_load(*nxt)
                    xt = xts[(b, tr)]
                    ts0 = 0
                    qps = psA.tile([P, IR], fp32, tag="pj", name="qps")
                    for e in range(ET):
                        nc.tensor.matmul(qps[:], wq_sb[:, e, :],
                                         xt[:, e, ts0:ts0 + IR],
                                         start=(e == 0), stop=(e == ET - 1))
                    nc.scalar.activation(
                        qT[b][:, tr * IR:(tr + 1) * IR], qps[:],
                        AF.Identity, bias=bq_sb[:])
                    kps = psA.tile([P, IR], fp32, tag="pj", name="kps")
                    for e in range(ET):
                        nc.tensor.matmul(kps[:], wk_sb[:, e, :],
                                         xt[:, e, ts0:ts0 + IR],
                                         start=(e == 0), stop=(e == ET - 1))
                    nc.scalar.activation(
                        kT[b][:, MXL + tr * IR:MXL + (tr + 1) * IR],
                        kps[:], AF.Identity, bias=bk_sb[:])
                    vps = psA.tile([P, IR], fp32, tag="pj", name="vps")
                    for e in range(ET):
                        nc.tensor.matmul(vps[:], wv_sb[:, e, :],
                                         xt[:, e, ts0:ts0 + IR],
                                         start=(e == 0), stop=(e == ET - 1))
                    vTs = sb1.tile([P, IR], fp16, tag="vTs")
                    nc.scalar.activation(vTs[:], vps[:],
                                         AF.Identity, bias=bv_sb[:])
                    for s in range(IR // P):
                        t0 = tr * IR + s * P
                        jt = MXL // P + t0 // P
                        vtp = psA.tile([P, P], fp16, tag="pj", name="vtp")
                        nc.tensor.transpose(vtp[:], vTs[:, s * P:(s + 1) * P],
                                            ident[:])
                        vsb = sb1.tile([P, P], fp32, tag="vsb")
                        nc.scalar.copy(vsb[:], vtp[:])
                        nc.vector.tensor_copy(vpv[b][:, jt, 0:D], vsb[:, 0:D])
                        nc.vector.tensor_copy(vpv[b][:, jt, PVW:PVW + D],
                                              vsb[:, D:2 * D])
                        nc.gpsimd.dma_start(
                            kvp.ap()[b * T + t0:b * T + t0 + P, 1, :], vsb[:])
                        ktp = psA.tile([P, P], fp16, tag="pj", name="ktp")
                        nc.tensor.transpose(
                            ktp[:], kT[b][:, MXL + t0:MXL + t0 + P], ident[:])
                        ksb = sb1.tile([P, P], fp32, tag="ksb")
                        nc.vector.tensor_copy(ksb[:], ktp[:])
                        nc.gpsimd.dma_start(
                            kvp.ap()[b * T + t0:b * T + t0 + P, 0, :], ksb[:])

            def attn(b):
                for ir in range(T // IR):
                    i0 = ir * IR
                    nj = (MXL + i0 + IR) // P
                    pvps = [psPV.tile([P, IR], fp32, tag=f"pv{h}",
                                      name=f"pv{h}", bufs=1)
                            for h in range(HPC)]
                    for jp in range(nj // 2):
                        sps = [psA.tile([P, 2, IR], fp32, tag=f"s{h}",
                                        name=f"s{h}", bufs=1)
                               for h in range(HPC)]
                        for u in range(2):
                            jt = jp * 2 + u
                            for h in range(HPC):
                                h0 = h * D
                                nc.tensor.matmul(
                                    sps[h][:, u, :],
                                    kT[b][h0:h0 + D, jt * P:(jt + 1) * P],
                                    qT[b][h0:h0 + D, i0:i0 + IR],
                                    start=True, stop=True)
                        for h in range(HPC):
                            exs = exsp.tile([P, 2, IR], fp16, tag="exs")
                            nc.scalar.activation(exs[:], sps[h][:], AF.Exp,
                                                 scale=SCALE)
                            pex = pexp.tile([P, 2, IR], fp16, tag="pex")
                            nc.vector.tensor_mul(
                                pex[:], exs[:],
                                er_sb[:, h, jp * 2:jp * 2 + 2, i0:i0 + IR])
                            for u in range(2):
                                jt = jp * 2 + u
                                nc.tensor.matmul(
                                    pvps[h][:],
                                    vpv[b][:, jt, h * PVW:(h + 1) * PVW],
                                    pex[:, u, :],
                                    start=(jt == 0), stop=(jt == nj - 1))
                    for h in range(HPC):
                        h0 = h * D
                        rs = smp.tile([D, IR], fp32, tag="rs")
                        nc.vector.tensor_copy(rs[:], pvps[h][D:2 * D, :])
                        rb = smp.tile([D, IR], fp32, tag="rb")
                        nc.vector.reciprocal_approx_fast(rb[:], rs[:])
                        nc.vector.tensor_mul(
                            qkvn[b][h0:h0 + D, i0:i0 + IR],
                            pvps[h][0:D, :], rb[:])
                    for tt in range(IR // P):
                        t0 = i0 + tt * P
                        for eh in range(E // 512):
                            ops = psA.tile([P, 512], fp32, tag="pj",
                                           name="ops")
                            nc.tensor.matmul(
                                ops[:], qkvn[b][:, t0:t0 + P],
                                wo_sb[:, eh * 512:(eh + 1) * 512],
                                start=True, stop=True)
                            osb = osbp.tile([P, 512], fp32, tag="osb")
                            nc.vector.tensor_copy(osb[:], ops[:])
                            nc.sync.dma_start(
                                outp.ap()[b * T + t0:b * T + t0 + P,
                                          eh * 512:(eh + 1) * 512],
                                osb[:])

            proj(0)
            feeds()
            proj(1)
            er_load(1)
            attn(0)
            proj(2)
            attn(1)
            proj(3)
            attn(2)
            attn(3)

    nc.compile()
    return nc


def _get_program():
    if "nc" not in _CACHE:
        _CACHE["nc"] = _build_program()
    return _CACHE["nc"]


def _prep_inputs(x, xl, rel, Wq, bq, Wk, bk, Wv, bv, Wo):
    """Host-side sharding/layout prep. Returns per-core input maps."""
    f16 = np.float16
    xT = np.ascontiguousarray(x.reshape(BT, E).T).astype(f16)   # [E, BT]

    # mask (j >= i + MXL + 1) and rel bias folded into exp(rel * SCALE)
    jj = np.arange(J, dtype=np.int64)[:, None]
    ii = np.arange(T, dtype=np.int64)[None, :]
    maskT = jj >= (ii + MXL + 1)                                # [J, T]

    in_maps = []
    for c in range(NCORES):
        cs = slice(c * HDC, (c + 1) * HDC)
        relc = np.exp(rel[c * HPC:(c + 1) * HPC].transpose(0, 2, 1) * SCALE)
        relc[:, maskT] = 0.0
        # [B, P, MXL//P, 2*PVW]: per-j-tile rows [vA | 1s | vB | 1s]
        va = xl[:, :, 1, cs].reshape(B, MXL // P, P, HPC, D)
        va = va.transpose(0, 2, 1, 3, 4)             # [B, P, jt, h, D]
        vxlq = np.ones((B, P, MXL // P, HPC * PVW), np.float16)
        vxlq[:, :, :, 0:D] = va[:, :, :, 0]
        vxlq[:, :, :, PVW:PVW + D] = va[:, :, :, 1]
        in_maps.append({
            "xTd": xT,
            "wq": np.ascontiguousarray(Wq[:, cs] * SCALE).astype(f16),
            "wk": np.ascontiguousarray(Wk[:, cs]).astype(f16),
            "wv": np.ascontiguousarray(Wv[:, cs]).astype(f16),
            "wo": np.ascontiguousarray(Wo[cs, :]).astype(f16),
            "bqd": np.ascontiguousarray(
                (bq[cs] * SCALE).reshape(HDC, 1).astype(np.float32)),
            "bkd": np.ascontiguousarray(bk[cs].reshape(HDC, 1)),
            "bvd": np.ascontiguousarray(bv[cs].reshape(HDC, 1)),
            "kxlT": np.ascontiguousarray(
                xl[:, :, 0, cs].reshape(BT, HDC).T).astype(f16),
            "vxlq": vxlq,
            "erel": np.ascontiguousarray(relc).astype(f16),
        })
    return in_maps


def _run(inputs, trace=False, tmpdir=None, trace_cores=None):
    from concourse.bass_utils import run_bass_kernel_spmd

    f = lambda k: np.asarray(inputs[k], np.float32)
    in_maps = _prep_inputs(f("x"), f("xl_memory"), f("relative_positions"),
                           f("Wq"), f("bq"), f("Wk"), f("bk"),
                           f("Wv"), f("bv"), f("Wo"))
    bo = f("bo")

    nc = _get_program()
    kw = {}
    if trace:
        kw.update(trace=True, tmpdir=tmpdir, trace_cores=trace_cores)
    res = run_bass_kernel_spmd(nc, in_maps, list(range(NCORES)), **kw)

    out = np.zeros((BT, E), np.float32)
    kv = np.empty((B, T, 2, H * D), np.float32)
    for c in range(NCORES):
        cs = slice(c * HDC, (c + 1) * HDC)
        out += res.results[c]["outp"]
        kv[:, :, :, cs] = res.results[c]["kvp"].reshape(B, T, 2, HDC)
    out = out.reshape(B, T, E) + bo
    return (out, kv), res


def kernel(**inputs):
    outs, _ = _run(inputs, trace=False)
    return outs
